# revision 2
# baseline (speedup 1.0000x reference)
"""Trainium2 Bass kernel for nn_BispectrumPool — Gauss-8 redesign.

Factorization (validated in check_math.py):
  p = U*V products: [b1^2, b2^2, b1b2, b1b5, b2b6, b2b5, b1b6]
  C-combines (PE): S, Gr+Gi, Gr, Gi, Hr, Hi-Hr, Kr, Hr+Hi (+Ki via Cb)
  t1 = Ca (.) asb, asb = [b0, b3, b3+b4, b3-b4, b3+b4, b3, b7, b4]
     = [f0, k1G, k2G, k3G, k1H, k2H, f5, k3H]  (Gauss 3-mult complex prods)
  f6 = b7*Ki in-place in R-PSUM (Cb writes Ki at rows 96:112, DVE RMW by b7)
  features = relu(ln(beta + 1)) == ln(1+relu(beta)) (ACT Ln yields NaN for
  x<=0; Pool max flushes NaN to 0 — verified on HW in probe_ln.py)
  conv bias added on host.

Per q (16ch x 448px): PE 7 matmuls (U,V,A,Ca,Cb,R1,conv; R1+Cb col-tiled),
DVE 3 (m1, t1, t2-RMW), ACT 3 (V-evac, A-evac, ln) + y-evac/chunk,
Pool 1 (relu). One x DMA per chunk (partition-major DRAM layout).

Distribution: pure data parallel, batch 16 -> 2 per core on 8 cores.
"""

import numpy as np

C, G = 64, 8
HWP = 56 * 56            # 3136
S = 448
NCHUNK = HWP // S        # 7
NCORES = 8
BPC = 2
NQ = 4


def _rows():
    g = np.arange(G)
    B0 = np.ones(G)
    B1 = np.cos(2 * np.pi * g / G); B2 = -np.sin(2 * np.pi * g / G)
    B3 = np.cos(4 * np.pi * g / G); B4 = -np.sin(4 * np.pi * g / G)
    B5 = np.cos(6 * np.pi * g / G); B6 = -np.sin(6 * np.pi * g / G)
    B7 = np.cos(np.pi * g)
    U = np.stack([B1, B2, B1, B1, B2, B2, B1])                       # 7
    V = np.stack([B1, B2, B2, B5, B6, B5, B6])                       # 7
    # b7 at block 6 (rows 96:112) so the t2 operand slice is 32-aligned
    A = np.stack([B0, B3, B3 + B4, B3 - B4, B3 + B4, B3, B7, B4])    # 8
    return U, V, A


def _combines():
    # p = [b1^2, b2^2, b1b2, b1b5, b2b6, b2b5, b1b6]
    Wc = np.zeros((8, 7))
    Wc[0, 0] = 1; Wc[0, 1] = 1                                   # S
    Wc[1, 0] = 1; Wc[1, 1] = -1; Wc[1, 2] = 2                    # Gr+Gi
    Wc[2, 0] = 1; Wc[2, 1] = -1                                  # Gr
    Wc[3, 2] = 2                                                 # Gi
    Wc[4, 3] = 1; Wc[4, 4] = 1                                   # Hr
    Wc[5, 5] = 1; Wc[5, 6] = -1; Wc[5, 3] = -1; Wc[5, 4] = -1    # Hi-Hr
    Wc[6, 3] = 1; Wc[6, 4] = -1                                  # Kr (pairs b7)
    Wc[7, 3] = 1; Wc[7, 4] = 1; Wc[7, 5] = 1; Wc[7, 6] = -1     # Hr+Hi
    Wcb = np.zeros((1, 7))
    Wcb[0, 5] = 1; Wcb[0, 6] = 1                                 # Ki
    # t1 = [f0, k1G, k2G, k3G, k1H, k2H, f5, k3H]
    Wr = np.zeros((6, 8))
    Wr[0, 0] = 1                                                 # f0
    Wr[1, 1] = 1; Wr[1, 3] = -1                                  # f1
    Wr[2, 1] = 1; Wr[2, 2] = -1                                  # f2
    Wr[3, 4] = 1; Wr[3, 7] = -1                                  # f3
    Wr[4, 4] = 1; Wr[4, 5] = 1                                   # f4
    Wr[5, 6] = 1                                                 # f5
    return Wc, Wcb, Wr


def _form_lhsT(rows):
    # input partitions (16c x 8g) c-major; out blocks of 16ch per form row
    n = rows.shape[0]
    lhsT = np.zeros((128, n * 16), dtype=np.float64)
    for j in range(n):
        for c in range(16):
            for g in range(G):
                lhsT[c * G + g, j * 16 + c] = rows[j, g]
    return lhsT


def _block_lhsT(coef, n_in):
    n_out = coef.shape[0]
    lhsT = np.zeros((n_in * 16, n_out * 16), dtype=np.float64)
    for o in range(n_out):
        for u in range(n_in):
            if coef[o, u] != 0.0:
                for c in range(16):
                    lhsT[u * 16 + c, o * 16 + c] = coef[o, u]
    return lhsT


def _build_consts():
    U, V, A = _rows()
    Wc, Wcb, Wr = _combines()
    cU = _form_lhsT(U).astype(np.float32)          # [128, 112]
    cV = _form_lhsT(V).astype(np.float32)          # [128, 112]
    cA = _form_lhsT(A).astype(np.float32)          # [128, 128]
    cCa = _block_lhsT(Wc, 7).astype(np.float32)    # [112, 128]
    # Cb widened to M=112: cols 0:96 zero, 96:112 = Ki-map. It start=True
    # writes the whole R tile (zeros + Ki); R1 then accumulates rows 0:96.
    cCb = np.zeros((112, 112), dtype=np.float32)
    cCb[:, 96:112] = _block_lhsT(Wcb, 7).astype(np.float32)
    cR1 = _block_lhsT(Wr, 8).astype(np.float32)    # [128, 96]
    return cU, cV, cA, cCa, cCb, cR1


def _fold_weights(conv_w):
    w = conv_w.reshape(64, C, 16).astype(np.float64)
    W7 = np.zeros((64, C, 7))
    W7[..., 0] = w[..., 0] + w[..., 7]
    W7[..., 1] = w[..., 1] + w[..., 6]
    W7[..., 2] = w[..., 9] + w[..., 14]
    W7[..., 3] = w[..., 2] + w[..., 5]
    W7[..., 4] = w[..., 10] + w[..., 13]
    W7[..., 5] = w[..., 3] + w[..., 4]
    W7[..., 6] = w[..., 11] + w[..., 12]
    wf = np.zeros((112, NQ * 64), dtype=np.float64)
    for q in range(NQ):
        for f in range(7):
            for cl in range(16):
                wf[f * 16 + cl, q * 64:(q + 1) * 64] = W7[:, q * 16 + cl, f]
    import ml_dtypes
    return wf.astype(ml_dtypes.bfloat16)


_PROG_CACHE = {}


def _build_program(loop_n=1):
    import concourse.bass as bass
    import concourse.bacc as bacc
    import concourse.tile as tile
    import concourse.mybir as mybir

    f32 = mybir.dt.float32
    f32r = mybir.dt.float32r
    nc = bacc.Bacc("TRN2", target_bir_lowering=False, debug=False,
                   num_devices=NCORES)

    # x partition-major: [b, partition(16c x 8g), q, pixel] so one DMA per
    # chunk lands 4 q-tiles side by side in the free dim
    x_d = nc.dram_tensor("x", [BPC, 128, NQ, HWP], f32r,
                         kind="ExternalInput").ap()
    cU_d = nc.dram_tensor("cU", [128, 112], f32r, kind="ExternalInput").ap()
    cV_d = nc.dram_tensor("cV", [128, 112], f32r, kind="ExternalInput").ap()
    cA_d = nc.dram_tensor("cA", [128, 128], f32r, kind="ExternalInput").ap()
    cCa_d = nc.dram_tensor("cCa", [112, 128], f32r, kind="ExternalInput").ap()
    cCb_d = nc.dram_tensor("cCb", [112, 112], f32r, kind="ExternalInput").ap()
    cR1_d = nc.dram_tensor("cR1", [128, 96], f32r, kind="ExternalInput").ap()
    wf_d = nc.dram_tensor("wf", [112, NQ * 64], mybir.dt.bfloat16,
                          kind="ExternalInput").ap()
    y_d = nc.dram_tensor("y", [BPC, 64, HWP], f32, kind="ExternalOutput").ap()

    LN = mybir.ActivationFunctionType.Ln
    MAX = mybir.AluOpType.max
    bf16 = mybir.dt.bfloat16

    with tile.TileContext(nc) as tc:
        with (
            tc.tile_pool(name="consts", bufs=1) as cpool,
            tc.tile_pool(name="xin", bufs=3) as xpool,
            tc.tile_pool(name="vsbp", bufs=3) as vsbp,
            tc.tile_pool(name="asbp", bufs=3) as asbp,
            tc.tile_pool(name="m1p", bufs=3) as m1p,
            tc.tile_pool(name="t1p", bufs=3) as t1p,
            tc.tile_pool(name="rlnp", bufs=3) as rlnp,
            tc.tile_pool(name="rlup", bufs=3) as rlup,
            tc.tile_pool(name="yout", bufs=2) as ypool,
            tc.tile_pool(name="psU", bufs=2, space="PSUM") as psU,
            tc.tile_pool(name="psV", bufs=1, space="PSUM") as psV,
            tc.tile_pool(name="psA", bufs=1, space="PSUM") as psA,
            tc.tile_pool(name="psCa", bufs=1, space="PSUM") as psCa,
            tc.tile_pool(name="psR", bufs=2, space="PSUM") as psR,
            tc.tile_pool(name="psY", bufs=1, space="PSUM") as psY,
        ):
            cU = cpool.tile([128, 112], f32r, tag="cU")
            cV = cpool.tile([128, 112], f32r, tag="cV")
            cA = cpool.tile([128, 128], f32r, tag="cA")
            cCa = cpool.tile([112, 128], f32r, tag="cCa")
            cCb = cpool.tile([112, 112], f32r, tag="cCb")
            cR1 = cpool.tile([128, 96], f32r, tag="cR1")
            wf = cpool.tile([112, NQ * 64], bf16, tag="wf")
            for t, d in [(cU, cU_d), (cV, cV_d), (cA, cA_d), (cCa, cCa_d),
                         (cCb, cCb_d), (cR1, cR1_d), (wf, wf_d)]:
                nc.sync.dma_start(out=t[:], in_=d)

            import contextlib
            loop_cm = (tc.For_i(0, loop_n, 1) if loop_n > 1
                       else contextlib.nullcontext())
            with loop_cm:
              for b in range(BPC):
                for j in range(NCHUNK):
                    s0 = j * S
                    pY = psY.tile([64, S], f32, tag="y")
                    # one DMA brings all 4 q-tiles: [128, NQ, S]
                    xt = xpool.tile([128, NQ, S], f32r, tag="x")
                    nc.sync.dma_start(out=xt[:, :, :],
                                      in_=x_d[b, :, :, s0:s0 + S])
                    for q in range(NQ):
                        xq = xt[:, q, :]
                        pV = psV.tile([112, S], f32, tag="v")
                        pU = psU.tile([112, S], f32, tag="u")
                        pA = psA.tile([128, S], f32, tag="a")
                        nc.tensor.matmul(pV[:], cV[:], xq)
                        nc.tensor.matmul(pU[:], cU[:], xq)
                        nc.tensor.matmul(pA[:], cA[:], xq)
                        vsb = vsbp.tile([112, S], f32r, tag="vsb")
                        asb = asbp.tile([128, S], f32r, tag="asb")
                        nc.scalar.copy(vsb[:], pV[:])
                        nc.scalar.copy(asb[:], pA[:])
                        m1 = m1p.tile([112, S], f32r, tag="m1")
                        nc.vector.tensor_mul(m1[:], pU[:], vsb[:])
                        pCa = psCa.tile([128, S], f32, tag="ca")
                        nc.tensor.matmul(pCa[:], cCa[:], m1[:])
                        t1 = t1p.tile([128, S], f32r, tag="t1")
                        nc.vector.tensor_mul(t1[:], pCa[:], asb[:])
                        pR = psR.tile([112, S], f32, tag="r")
                        nc.tensor.matmul(pR[0:112, :], cCb[:], m1[:],
                                         start=True, stop=False,
                                         skip_group_check=True)
                        nc.tensor.matmul(pR[0:96, :], cR1[:], t1[:],
                                         start=False, stop=True,
                                         skip_group_check=True)
                        nc.vector.tensor_mul(pR[96:112, :], pR[96:112, :],
                                             asb[96:112, :])
                        rln = rlnp.tile([112, S], bf16, tag="rln")
                        nc.scalar.activation(rln[:], pR[:], LN, bias=1.0)
                        rlu = rlup.tile([112, S], bf16, tag="rlu")
                        nc.vector.tensor_scalar(rlu[:], rln[:], 0.0, None,
                                                MAX)
                        nc.tensor.matmul(pY[:], wf[:, q * 64:(q + 1) * 64],
                                         rlu[:],
                                         start=(q == 0), stop=(q == NQ - 1))
                    yt = ypool.tile([64, S], f32, tag="yt")
                    nc.scalar.copy(yt[:], pY[:])
                    nc.sync.dma_start(out=y_d[b, :, s0:s0 + S], in_=yt[:])
    nc.compile()
    return nc


def _make_in_maps(x, conv_w):
    cU, cV, cA, cCa, cCb, cR1 = _build_consts()
    wf = _fold_weights(np.asarray(conv_w))
    B = x.shape[0]
    # [B, C, G, H, W] -> [B, q, c, g, hw] -> [B, (c g)=128, q, hw]
    xr = np.asarray(x).reshape(B, NQ, 16, G, HWP)
    xr = np.ascontiguousarray(
        xr.transpose(0, 2, 3, 1, 4).reshape(B, 128, NQ, HWP)
    ).astype(np.float32)
    consts = dict(cU=cU, cV=cV, cA=cA, cCa=cCa, cCb=cCb, cR1=cR1, wf=wf)
    in_maps = []
    for i in range(NCORES):
        m = dict(consts)
        m["x"] = np.ascontiguousarray(xr[i * BPC:(i + 1) * BPC])
        in_maps.append(m)
    return in_maps


def kernel(x, conv_w, conv_b):
    from concourse.bass_utils import run_bass_kernel_spmd

    x = np.asarray(x)
    conv_w = np.asarray(conv_w)
    conv_b = np.asarray(conv_b, dtype=np.float32)
    B = x.shape[0]

    key = "prog"
    if key not in _PROG_CACHE:
        _PROG_CACHE[key] = _build_program()
    nc = _PROG_CACHE[key]

    in_maps = _make_in_maps(x, conv_w)
    res = run_bass_kernel_spmd(nc, in_maps, core_ids=list(range(NCORES)))
    y = np.concatenate([res.results[i]["y"] for i in range(NCORES)], axis=0)
    y = y.reshape(B, 64, 56, 56).astype(np.float32)
    y += conv_b[None, :, None, None]
    return np.ascontiguousarray(y)
